# revision 32
# baseline (speedup 1.0000x reference)
"""Trainium2 Bass kernel for nn_ContactPredictionHead.

Reference computation (B=2, L=2048, D=1536, T=2):
    Wp, Wd = W[:, :D], W[:, D:]
    prod[b,i,j,t] = sum_d h[b,i,d] * Wp[t,d] * h[b,j,d]
    diff[b,i,j,t] = (h@Wd.T)[b,i,t] - (h@Wd.T)[b,j,t]
    out = symmetrize(prod + diff + bias)

Key identity: prod is symmetric in (i,j) and diff is antisymmetric, so the
symmetrization leaves   out[b,i,j,t] = prod[b,i,j,t] + bias[t]   exactly —
a weighted Gram matrix, 4 big matmuls ([2048,1536] @ [1536,2048]).

Symmetry is exploited: only the block-upper-triangle of each L x L Gram
matrix is computed on device; the host mirrors the strict lower triangle.

Sharding: 4 cores per batch item.  Core cc (0..3) of a batch receives
ht = roll(h[b].T, -128*cc, axis=1).  In this rotated frame every core runs
the SAME program: its stationary row-slots s=0..3 are local columns
[512s, 512s+128) (global rows 512s + 128cc — so slot s on the 4 cores
covers global row blocks 4s+cc, a balanced interleave of the triangle),
and it computes blocks (s, v) for v >= s against local j-chunks v.  In
global terms each row gets every j >= i covered (the v-arc from the row's
own slot wraps around), so the host can fill j < i by transposition.

The matmuls run in float32r (full-rate fp32 on the PE array, ~1e-4 rel err).
"""
import sys

sys.path.insert(0, "/opt/trn_rl_repo")

import numpy as np

B, L, D, T = 2, 2048, 1536, 2
NCORES = 8
CPB = NCORES // B     # cores per batch item = 4
NK = D // 128         # contraction k-tiles = 12
NKH = NK // 2         # k-tiles per DMA half
NJ = 512              # j columns per matmul (one PSUM bank of fp32)
NNB = L // NJ         # j chunks = 4
NS = 4                # stationary row slots per core (128 rows each)
BLOCKS = [(s, v) for v in range(NNB) for s in range(v + 1)]   # 10 blocks

_CACHE = {}


def _get_nc():
    if "nc" in _CACHE:
        return _CACHE["nc"]
    import concourse.tile as tile
    from concourse.tile_rust import add_dep_helper
    from concourse import bacc, mybir

    f32, f32r = mybir.dt.float32, mybir.dt.float32r
    nc = bacc.Bacc("TRN2", target_bir_lowering=False, debug=False,
                   num_devices=NCORES, enable_partition_id=False,
                   enable_asserts=False)
    ht_d = nc.dram_tensor("ht", [D, L], f32r, kind="ExternalInput")
    wp_d = nc.dram_tensor("wp", [128, T * NK], f32, kind="ExternalInput")
    out_d = nc.dram_tensor("out", [len(BLOCKS), T, 128, NJ], f32,
                           kind="ExternalOutput")

    with tile.TileContext(nc) as tc:
        with tc.tile_pool(name="big", bufs=1) as big, \
             tc.tile_pool(name="st", bufs=4) as stp, \
             tc.tile_pool(name="ps", bufs=4, space="PSUM") as psp:
            # Allocation order fixes SBUF addresses: stationary operands low,
            # moving operands above them (measured faster LDWEIGHTS).
            wt = big.tile([128, T * NK], f32, name="wt")
            # a[t][p, s, k, x] = ht_local[128k+p, 512s+x] * Wp[t, 128k+p]
            a = [big.tile([128, NS, NK, 128], f32r, name=f"a{t}")
                 for t in range(T)]
            # htn[v][p, k*NJ + j] = ht_local[128k+p, 512v + j]
            htn = [big.tile([128, NK * NJ], f32r, name=f"htn{v}")
                   for v in range(NNB)]

            # wt arrives pre-gathered from the host: wp[p, t*NK+k]
            # = Wp[t, 128k+p], so this DMA is a contiguous identity copy.
            nc.scalar.dma_start(wt[:], wp_d.ap())
            # Warm the PE clock (HAM un-throttles after ~3.4 us of activity)
            # with throwaway matmuls on a locally-initialized scratch tile —
            # no DMA dependency, so warmup starts during the preamble.
            wdum = big.tile([128, 128], mybir.dt.bfloat16, name="wdum")
            nc.vector.memset(wdum[:], 0.0)
            wacc = psp.tile([128, 128], f32, name="wacc")
            for _ in range(48):
                nc.tensor.matmul(wacc[:], wdum[:], wdum[:], start=True,
                                 stop=True)
            # ht chunks land in fixed order so the PE can start on chunk 0
            # while the rest stream in.  Parts alternate between the two
            # HWDGE rings (sync / scalar) to raise aggregate DMA bandwidth;
            # chunk 0 is quartered for an earlier first matmul.
            prev = None
            for v in range(NNB):
                nparts = 4 if v == 0 else 2
                kq = NK // nparts
                for h in range(nparts):
                    dma = nc.sync.dma_start(
                        htn[v][:, h * kq * NJ:(h + 1) * kq * NJ]
                        .rearrange("p (k j) -> p k j", k=kq),
                        ht_d.ap()[h * kq * 128:(h + 1) * kq * 128,
                                  v * NJ:(v + 1) * NJ]
                        .rearrange("(k p) j -> p k j", p=128))
                    if prev is not None:
                        add_dep_helper(dma.ins, prev.ins, sync=False,
                                       reason="ht chunks stream in j order")
                    prev = dma

            # Stationary operands: muls split to match the chunk-s DMA parts,
            # reading columns 0:128 of every k-block of htn[s]; the per-(p,k)
            # scale comes from a stride-0 broadcast of wt along x.
            for s in range(NS):
                src = htn[s][:].rearrange("p (k j) -> p k j", k=NK)
                nparts = 4 if s == 0 else 2
                kq = NK // nparts
                for h in range(nparts):
                    ks = slice(h * kq, (h + 1) * kq)
                    for t in range(T):
                        scale = (wt[:, t * NK + h * kq: t * NK + (h + 1) * kq]
                                 .unsqueeze(2).broadcast_to([128, kq, 128]))
                        nc.vector.tensor_mul(
                            a[t][:, s, ks], src[:, ks, 0:128], scale)

            for bi, (s, v) in enumerate(BLOCKS):
                for t in range(T):
                    acc = psp.tile([128, NJ], f32, name="acc", tag="acc")
                    for k in range(NK):
                        nc.tensor.matmul(
                            acc[:], a[t][:, s, k], htn[v][:, k * NJ:(k + 1) * NJ],
                            start=(k == 0), stop=(k == NK - 1))
                    st = stp.tile([128, NJ], f32, name="st", tag="st")
                    if t == 0:
                        nc.vector.tensor_copy(st[:], acc[:])
                    else:
                        nc.scalar.copy(st[:], acc[:])
                    out_eng = nc.scalar if bi >= len(BLOCKS) - 2 else nc.gpsimd
                    out_eng.dma_start(out_d.ap()[bi, t], st[:])
                if bi == 0:
                    # Minimal activity to keep the HAM monitor from
                    # re-throttling the PE clock during the chunk-1 wait
                    # (measured 3.4 us at half clock without this).
                    for _ in range(8):
                        nc.tensor.matmul(wacc[0:64, 0:64], wdum[:, 0:64],
                                         wdum[:, 0:64], start=True, stop=True)
    nc.compile()
    _CACHE["nc"] = nc
    return nc


def make_in_maps(h, W):
    # wp[p, t*NK+k] = Wp[t, 128k+p]
    wp = np.ascontiguousarray(
        W[:, :D].reshape(T, NK, 128).transpose(2, 0, 1).reshape(128, T * NK))
    hts = [np.ascontiguousarray(h[bi].T) for bi in range(B)]   # [D, L]
    in_maps = []
    for c in range(NCORES):
        bi, r = c // CPB, (c % CPB) * 128
        ht = hts[bi] if r == 0 else np.roll(hts[bi], -r, axis=1)
        in_maps.append({"ht": np.ascontiguousarray(ht), "wp": wp})
    return in_maps


def kernel(hidden_states, W, b):
    from concourse.bass_utils import run_bass_kernel_spmd

    h = np.ascontiguousarray(hidden_states, dtype=np.float32)
    W = np.asarray(W, dtype=np.float32)
    bias = np.asarray(b, dtype=np.float32)
    nc = _get_nc()

    res = run_bass_kernel_spmd(nc, make_in_maps(h, W),
                               core_ids=list(range(NCORES)))
    full = np.empty((B, L, L, T), np.float32)
    for c in range(NCORES):
        bi, r = c // CPB, (c % CPB) * 128
        # [len(BLOCKS), T, 128, NJ] -> [len(BLOCKS), 128, NJ, T]
        blocks = res.results[c]["out"].transpose(0, 2, 3, 1)
        for idx, (s, v) in enumerate(BLOCKS):
            rows = slice(512 * s + r, 512 * s + r + 128)
            g = (512 * v + r) % L
            blk = blocks[idx]
            if g + NJ <= L:
                full[bi, rows, g:g + NJ] = blk
            else:
                w = L - g
                full[bi, rows, g:] = blk[:, :w]
                full[bi, rows, :NJ - w] = blk[:, w:]
    # Mirror: keep computed j >= i, take j < i from the transpose.
    idx = np.arange(L)
    mask = (idx[None, :] >= idx[:, None])[None, :, :, None]
    out = np.where(mask, full, full.transpose(0, 2, 1, 3))
    if np.any(bias != 0):
        out += bias
    return out
